# revision 5
# baseline (speedup 1.0000x reference)
"""Trainium2 Bass kernel for nn_MultiHeadAttention_76587856823057.

Sharding: (batch, query-half) -> 8 cores, zero collectives.
Per core: b fixed, queries TQ=1024 (half of T), all H=16 heads, all TK=2048 keys.

v2 design notes:
 - softmax is shift-invariant; the reference's *global* max subtract cancels in
   the normalization; scores are bounded so exp() cannot overflow in fp32.
 - exp(s*m)*m == exp(s)*m for m in {0,1}: one mask multiply only (after exp).
 - QK contracts K=64 per head using PE row-tiling (tile_position): head-even on
   array rows 0-63, head-odd on rows 64-127, running CONCURRENTLY -> ~2x QK.
 - row sums come free from the PV matmul via a ones-column per head (M=65);
   normalization (1/rowsum) is fused into the PV evacuation via gpsimd
   partition_broadcast - no DRAM round trips.
 - attention output stays in SBUF (bf16) and feeds the O-projection as the
   STATIONARY operand, producing x in [q, d] layout directly -> LayerNorm needs
   no transposes at all.
 - all weights/activations bf16 (residual path fp32); biases folded in via
   ones-row K=1 matmuls.
 - K-projection and V-projection are interleaved into the ACT(exp)-bound
   attention phase to fill PE slack.
 - PSUM: qk pool 2x[128,1024] (4 banks) + pv 2x[65,512] (2) + aux 2x[128,512]
   (2) = 8 banks.

Self-contained: hardcodes all shapes; no sibling imports.
"""

import os
import numpy as np

import concourse.bass as bass
from concourse import bacc
import concourse.mybir as mybir
from concourse.tile import TileContext
from concourse.bass_utils import run_bass_kernel_spmd

F32 = mybir.dt.float32
BF16 = mybir.dt.bfloat16
AF = mybir.ActivationFunctionType

B, T, D, H, DK = 4, 2048, 1024, 16, 64
TQ = T // 2          # queries per core
TK = T               # keys per core
NCORES = 8
NPAIR = H // 2       # 8 head pairs
NFT = D // 128       # 8 feature tiles
NKT = TK // 128      # 16 key tiles
VEXT = H * (DK + 1)  # 1040: per-head [64 v-cols + ones col]

_LAST_RESULTS = {}


def build_program(nc: bass.Bass, trivial_affine: bool = False):
    # ---- per-core DRAM I/O ----
    qT = nc.dram_tensor("qT", [D + 1, TQ], BF16, kind="ExternalInput").ap()
    kT = nc.dram_tensor("kT", [D + 1, TK], BF16, kind="ExternalInput").ap()
    vT = nc.dram_tensor("vT", [D + 1, TK], BF16, kind="ExternalInput").ap()
    wq = nc.dram_tensor("wq", [D + 1, D], BF16, kind="ExternalInput").ap()
    wk = nc.dram_tensor("wk", [D + 1, D], BF16, kind="ExternalInput").ap()
    wv = nc.dram_tensor("wv", [D + 1, VEXT], BF16, kind="ExternalInput").ap()
    wo = nc.dram_tensor("wo", [D + 1, D], BF16, kind="ExternalInput").ap()
    maskT = nc.dram_tensor("maskT", [TK, TQ], BF16, kind="ExternalInput").ap()
    qres = nc.dram_tensor("qres", [TQ, D], F32, kind="ExternalInput").ap()
    gam = nc.dram_tensor("gam", [1, D], F32, kind="ExternalInput").ap()
    bet = nc.dram_tensor("bet", [1, D], F32, kind="ExternalInput").ap()
    out = nc.dram_tensor("out", [TQ, D], F32, kind="ExternalOutput").ap()

    with TileContext(nc) as tc:
        import contextlib
        with contextlib.ExitStack() as ctx:
            pers = ctx.enter_context(tc.tile_pool(name="pers", bufs=1))

            qhT = pers.tile([128, NFT, TQ], BF16)        # 16 KB/part
            vh_sb = pers.tile([128, NKT, VEXT], BF16)    # 32.5 KB/part
            mk = pers.tile([128, NKT, TQ], BF16)         # 32 KB/part
            attn_sb = pers.tile([128, NPAIR, TQ], BF16)  # 16 KB/part
            qT_b = pers.tile([1, TQ], BF16)              # ones row (bias mms)
            kT_b = pers.tile([1, TK], BF16)              # ones row

            # PSUM pools (8 banks total): qk 4 + pv 2 + aux 2
            # (bufs is per-tag: pvps holds tags pv0+pv1, 1 buf each)
            apsum = ctx.enter_context(contextlib.ExitStack())
            qkps = apsum.enter_context(
                tc.tile_pool(name="qkps", bufs=2, space="PSUM"))
            pvps = apsum.enter_context(
                tc.tile_pool(name="pvps", bufs=1, space="PSUM"))
            auxps = apsum.enter_context(
                tc.tile_pool(name="auxps", bufs=2, space="PSUM"))

            pepool = ctx.enter_context(tc.tile_pool(name="pepool", bufs=3))
            pmpool = ctx.enter_context(tc.tile_pool(name="pmpool", bufs=8))
            evpool = ctx.enter_context(tc.tile_pool(name="evpool", bufs=2))

            nc.sync.dma_start(out=qT_b, in_=qT[D:D + 1, :])
            nc.sync.dma_start(out=kT_b, in_=kT[D:D + 1, :])
            nc.sync.dma_start(
                out=mk, in_=maskT.rearrange("(t p) q -> p t q", p=128))

            # ---------------- Q projection -> qhT resident ------------------
            with tc.tile_pool(name="qppool", bufs=1) as qp:
                wq_m = qp.tile([128, NFT, D], BF16, tag="wq_m")
                wq_b = qp.tile([1, D], BF16, tag="wq_b")
                qT_m = qp.tile([128, NFT, TQ], BF16, tag="qT_m")
                nc.sync.dma_start(
                    out=wq_m, in_=wq[0:D, :].rearrange("(k p) f -> p k f", p=128))
                nc.sync.dma_start(out=wq_b, in_=wq[D:D + 1, :])
                nc.sync.dma_start(
                    out=qT_m, in_=qT[0:D, :].rearrange("(k p) t -> p k t", p=128))
                for fi in range(NFT):
                    fs = slice(fi * 128, (fi + 1) * 128)
                    for c in range(2):
                        cs = slice(c * 512, (c + 1) * 512)
                        ps = auxps.tile([128, 512], F32, tag="aux", name="aux")
                        for ki in range(NFT):
                            nc.tensor.matmul(ps, wq_m[:, ki, fs], qT_m[:, ki, cs],
                                             start=(ki == 0), stop=False)
                        nc.tensor.matmul(ps, wq_b[0:1, fs], qT_b[0:1, cs],
                                         start=False, stop=True)
                        if c == 0:
                            nc.scalar.copy(qhT[:, fi, cs], ps)
                        else:
                            nc.vector.tensor_copy(qhT[:, fi, cs], ps)

            mul_ctr = [0]

            def pair_stream(j, khp, interleave):
                """Emit QK/exp/mask/PV for pair j.  `interleave` is a list of
                zero-arg emitters (vproj tiles / kproj quarters) drained into
                PE slack inside this pair's stream."""
                h0sl = slice((2 * j) * 65, (2 * j) * 65 + 65)
                h1sl = slice((2 * j + 1) * 65, (2 * j + 1) * 65 + 65)
                for qh in range(2):
                    qsl = slice(qh * 512, (qh + 1) * 512)
                    pv0 = pvps.tile([65, 512], F32, tag="pv0", name="pv0")
                    pv1 = pvps.tile([65, 512], F32, tag="pv1", name="pv1")
                    work = []   # (pm0, pm1, kt)

                    def emit_pv():
                        pm0, pm1, kt = work.pop(0)
                        nc.tensor.matmul(pv0, vh_sb[:, kt, h0sl], pm0,
                                         start=(kt == 0), stop=(kt == NKT - 1))
                        nc.tensor.matmul(pv1, vh_sb[:, kt, h1sl], pm1,
                                         start=(kt == 0), stop=(kt == NKT - 1))

                    for kt in range(NKT):
                        tsl = slice(kt * 128, (kt + 1) * 128)
                        qk = qkps.tile([128, 1024], F32, tag="qk", name="qk")
                        # concurrent row-tiled QK: h-even rows 0-63, h-odd 64-127
                        nc.tensor.matmul(qk[:, 0:512], khp[0:64, tsl],
                                         qhT[0:64, j, qsl], start=True, stop=True)
                        nc.tensor.matmul(qk[:, 512:1024], khp[64:128, tsl],
                                         qhT[64:128, j, qsl], start=True, stop=True)
                        pe = pepool.tile([128, 1024], BF16, tag="pe", name="pe")
                        nc.scalar.activation(pe, qk, AF.Exp)
                        pm0 = pmpool.tile([128, 512], BF16, tag="pm", name="pm")
                        pm1 = pmpool.tile([128, 512], BF16, tag="pm", name="pm")
                        mul_ctr[0] += 1
                        if mul_ctr[0] % 4 == 0:
                            nc.gpsimd.tensor_mul(pm0, pe[:, 0:512], mk[:, kt, qsl])
                        else:
                            nc.vector.tensor_mul(pm0, pe[:, 0:512], mk[:, kt, qsl])
                        mul_ctr[0] += 1
                        if mul_ctr[0] % 4 == 0:
                            nc.gpsimd.tensor_mul(pm1, pe[:, 512:1024], mk[:, kt, qsl])
                        else:
                            nc.vector.tensor_mul(pm1, pe[:, 512:1024], mk[:, kt, qsl])
                        work.append((pm0, pm1, kt))
                        if interleave:
                            interleave.pop(0)()
                        if len(work) > 2:
                            emit_pv()
                    while work:
                        emit_pv()

                    # evacuate: normalize by row sums, write bf16 attn tile
                    rr = evpool.tile([1, 512], F32, tag="rr", name="rr")
                    nc.vector.reciprocal(rr[0:1, :], pv0[64:65, :])
                    rrb = evpool.tile([64, 512], F32, tag="rrb", name="rrb")
                    nc.gpsimd.partition_broadcast(rrb, rr[0:1, :])
                    nc.vector.tensor_mul(attn_sb[0:64, j, qsl], pv0[0:64, :], rrb)
                    rr1 = evpool.tile([1, 512], F32, tag="rr", name="rr")
                    nc.vector.reciprocal(rr1[0:1, :], pv1[64:65, :])
                    rrb1 = evpool.tile([64, 512], F32, tag="rrb", name="rrb")
                    nc.gpsimd.partition_broadcast(rrb1, rr1[0:1, :])
                    nc.vector.tensor_mul(attn_sb[64:128, j, qsl], pv1[0:64, :], rrb1)

            # ------------- attention phase with interleaved K/V proj --------
            with tc.tile_pool(name="kwpool", bufs=1) as kw, \
                 tc.tile_pool(name="kqpool", bufs=2) as kqpool, \
                 tc.tile_pool(name="khpool", bufs=2) as khpool:
                wk_m = kw.tile([128, NFT, D], BF16, tag="wk_m")
                wk_b = kw.tile([1, D], BF16, tag="wk_b")
                nc.sync.dma_start(
                    out=wk_m, in_=wk[0:D, :].rearrange("(k p) f -> p k f", p=128))
                nc.sync.dma_start(out=wk_b, in_=wk[D:D + 1, :])

                def emit_kproj_qtr(j, khp, qtr):
                    # khp[:, qtr] = (k @ Wk + bk).T rows j*128.., key qtr slice
                    qs = slice(qtr * 512, (qtr + 1) * 512)
                    kT_q = kqpool.tile([128, NFT, 512], BF16, tag="ktq", name="ktq")
                    nc.sync.dma_start(
                        out=kT_q,
                        in_=kT[0:D, qs].rearrange("(k p) t -> p k t", p=128))
                    ps = auxps.tile([128, 512], F32, tag="aux", name="aux")
                    fs = slice(j * 128, (j + 1) * 128)
                    for ki in range(NFT):
                        nc.tensor.matmul(ps, wk_m[:, ki, fs], kT_q[:, ki, :],
                                         start=(ki == 0), stop=False)
                    nc.tensor.matmul(ps, wk_b[0:1, fs], kT_b[0:1, qs],
                                     start=False, stop=True)
                    nc.vector.tensor_copy(khp[:, qs], ps)

                with tc.tile_pool(name="vwpool", bufs=1) as vw, \
                     tc.tile_pool(name="vstage", bufs=2) as vstage:
                    wv_m = vw.tile([128, NFT, VEXT], BF16, tag="wv_m")
                    wv_b = vw.tile([1, VEXT], BF16, tag="wv_b")
                    vT_b = vw.tile([1, TK], BF16, tag="vT_b")
                    nc.sync.dma_start(
                        out=wv_m, in_=wv[0:D, :].rearrange("(k p) f -> p k f", p=128))
                    nc.sync.dma_start(out=wv_b, in_=wv[D:D + 1, :])
                    nc.sync.dma_start(out=vT_b, in_=vT[D:D + 1, :])

                    VCH = [(0, 512), (512, 1024), (1024, VEXT)]

                    def emit_vproj_ti(ti):
                        # vh_sb[:, ti, :] = (v @ Wv_ext + bv_ext).T tile ti
                        tsl = slice(ti * 128, (ti + 1) * 128)
                        vT_m = vstage.tile([128, NFT, 128], BF16, tag="vT_m",
                                           name="vTm")
                        nc.sync.dma_start(
                            out=vT_m,
                            in_=vT[0:D, tsl].rearrange("(k p) t -> p k t", p=128))
                        for (c0, c1) in VCH:
                            ps = auxps.tile([128, 512], F32, tag="aux", name="aux")
                            n = c1 - c0
                            for ki in range(NFT):
                                nc.tensor.matmul(ps[:, 0:n], vT_m[:, ki, :],
                                                 wv_m[:, ki, c0:c1],
                                                 start=(ki == 0), stop=False)
                            nc.tensor.matmul(ps[:, 0:n], vT_b[0:1, tsl],
                                             wv_b[0:1, c0:c1],
                                             start=False, stop=True)
                            if c0 == 512:
                                nc.scalar.copy(vh_sb[:, ti, c0:c1], ps[:, 0:n])
                            else:
                                nc.vector.tensor_copy(vh_sb[:, ti, c0:c1],
                                                      ps[:, 0:n])

                    # pair 0: K-proj(0) first; V-proj interleaved into stream
                    khp0 = khpool.tile([128, TK], BF16, tag="khp", name="khp")
                    for qtr in range(4):
                        emit_kproj_qtr(0, khp0, qtr)
                    emit_vproj_ti(0)
                    emit_vproj_ti(1)

                    khp_next = khpool.tile([128, TK], BF16, tag="khp", name="khp")
                    il0 = [(lambda ti=ti: emit_vproj_ti(ti))
                           for ti in range(2, NKT)]
                    il0 += [(lambda q=q: emit_kproj_qtr(1, khp_next, q))
                            for q in range(4)]
                    pair_stream(0, khp0, il0)
                    assert not il0

                khp_cur = khp_next
                for j in range(1, NPAIR):
                    if j < NPAIR - 1:
                        khp_nx = khpool.tile([128, TK], BF16, tag="khp",
                                             name="khp")
                        il = [(lambda q=q, t=khp_nx, jj=j + 1:
                               emit_kproj_qtr(jj, t, q)) for q in range(4)]
                    else:
                        khp_nx, il = None, []
                    pair_stream(j, khp_cur, il)
                    assert not il
                    khp_cur = khp_nx

            # close attention PSUM pools before phase C needs its banks
            apsum.close()

            # ------------ phase C: out-proj + residual + LN -----------------
            with tc.tile_pool(name="cw", bufs=1) as cw, \
                 tc.tile_pool(name="cq", bufs=2) as cq, \
                 tc.tile_pool(name="cl", bufs=2) as cl, \
                 tc.tile_pool(name="cps", bufs=2, space="PSUM") as cps:

                wo_m = cw.tile([128, NFT, D], BF16)
                wo_b = cw.tile([1, D], BF16)
                nc.sync.dma_start(
                    out=wo_m, in_=wo[0:D, :].rearrange("(k p) f -> p k f", p=128))
                nc.sync.dma_start(out=wo_b, in_=wo[D:D + 1, :])
                eps_t = cw.tile([128, 1], F32)
                nc.vector.memset(eps_t, 1e-5)
                if not trivial_affine:
                    gam_r = cw.tile([1, D], F32)
                    bet_r = cw.tile([1, D], F32)
                    nc.sync.dma_start(out=gam_r, in_=gam)
                    nc.sync.dma_start(out=bet_r, in_=bet)
                    gam_b = cw.tile([128, D], F32)
                    bet_b = cw.tile([128, D], F32)
                    nc.gpsimd.partition_broadcast(gam_b, gam_r)
                    nc.gpsimd.partition_broadcast(bet_b, bet_r)

                for qt in range(NFT):
                    qts = slice(qt * 128, (qt + 1) * 128)
                    ps = cps.tile([128, D], F32, tag="x")
                    for c in range(2):
                        cs = slice(c * 512, (c + 1) * 512)
                        for ki in range(NFT):
                            nc.tensor.matmul(ps[:, cs], attn_sb[:, ki, qts],
                                             wo_m[:, ki, cs],
                                             start=(ki == 0), stop=False)
                        nc.tensor.matmul(ps[:, cs], qT_b[0:1, qts],
                                         wo_b[0:1, cs], start=False, stop=True)
                    qres_t = cq.tile([128, D], F32, tag="qres")
                    nc.sync.dma_start(out=qres_t, in_=qres[qts, :])
                    x_sb = cq.tile([128, D], F32, tag="x_sb")
                    nc.vector.tensor_add(x_sb, ps, qres_t)

                    stats = cl.tile([128, 2, 6], F32, tag="stats")
                    nc.vector.bn_stats(stats[:, 0, :], x_sb[:, 0:512])
                    nc.vector.bn_stats(stats[:, 1, :], x_sb[:, 512:1024])
                    mv = cl.tile([128, 2], F32, tag="mv")
                    nc.vector.bn_aggr(mv, stats)
                    sq = cl.tile([128, 1], F32, tag="sq")
                    nc.scalar.activation(sq, mv[:, 1:2], AF.Sqrt, bias=eps_t)
                    rstd = cl.tile([128, 1], F32, tag="rstd")
                    nc.vector.reciprocal(rstd, sq)
                    xo = cl.tile([128, D], F32, tag="xo")
                    nc.vector.tensor_scalar(xo, x_sb, mv[:, 0:1], rstd,
                                            op0=mybir.AluOpType.subtract,
                                            op1=mybir.AluOpType.mult)
                    if not trivial_affine:
                        nc.vector.tensor_mul(xo, xo, gam_b)
                        nc.vector.tensor_add(xo, xo, bet_b)
                    nc.sync.dma_start(out=out[qts, :], in_=xo)
    return nc


def _prep_core_inputs(inputs, b, qh):
    """Build the per-core input map (host-side layout prep only)."""
    import ml_dtypes
    bf = ml_dtypes.bfloat16
    q = np.asarray(inputs["q"], np.float32)
    k = np.asarray(inputs["k"], np.float32)
    v = np.asarray(inputs["v"], np.float32)
    mask = np.asarray(inputs["attn_mask"])
    Wq, bq = np.asarray(inputs["Wq"], np.float32), np.asarray(inputs["bq"], np.float32)
    Wk, bk = np.asarray(inputs["Wk"], np.float32), np.asarray(inputs["bk"], np.float32)
    Wv, bv = np.asarray(inputs["Wv"], np.float32), np.asarray(inputs["bv"], np.float32)
    Wo, bo = np.asarray(inputs["Wo"], np.float32), np.asarray(inputs["bo"], np.float32)
    gamma, beta = np.asarray(inputs["gamma"], np.float32), np.asarray(inputs["beta"], np.float32)

    qs = slice(qh * TQ, (qh + 1) * TQ)
    qb = q[b, qs, :]                       # [TQ, D]

    def ext_T(x_t):  # [D, N] -> [D+1, N] with ones row
        return np.concatenate([x_t, np.ones((1, x_t.shape[1]), np.float32)], axis=0)

    def ext_W(W, bias):  # [D, N] -> [D+1, N] with bias row
        return np.concatenate([W, bias[None, :]], axis=0)

    # Wv extended with per-head ones column: col h*65+64 gets bias 1, weights 0
    Wv_ext = np.zeros((D, VEXT), np.float32)
    bv_ext = np.zeros((VEXT,), np.float32)
    for h in range(H):
        Wv_ext[:, h * 65:h * 65 + 64] = Wv[:, h * 64:(h + 1) * 64]
        bv_ext[h * 65:h * 65 + 64] = bv[h * 64:(h + 1) * 64]
        bv_ext[h * 65 + 64] = 1.0

    return {
        "qT": ext_T(qb.T.copy()).astype(bf),
        "kT": ext_T(k[b].T.copy()).astype(bf),
        "vT": ext_T(v[b].T.copy()).astype(bf),
        "wq": ext_W(Wq, bq).astype(bf),
        "wk": ext_W(Wk, bk).astype(bf),
        "wv": ext_W(Wv_ext, bv_ext).astype(bf),
        "wo": ext_W(Wo, bo).astype(bf),
        "maskT": np.ascontiguousarray(mask[b, qs, :].T).astype(bf),
        "qres": np.ascontiguousarray(qb),
        "gam": gamma[None, :].copy(),
        "bet": beta[None, :].copy(),
    }


def kernel(**inputs) -> np.ndarray:
    global _LAST_RESULTS
    trivial_affine = (np.all(np.asarray(inputs["gamma"]) == 1.0)
                      and np.all(np.asarray(inputs["beta"]) == 0.0))
    nc = bacc.Bacc("TRN2", debug=False, num_devices=NCORES)
    build_program(nc, trivial_affine=trivial_affine)
    nc.finalize()

    ncores_run = int(os.environ.get("KERNEL_NCORES", str(NCORES)))
    in_maps = [_prep_core_inputs(inputs, c // 2, c % 2) for c in range(NCORES)]
    trace = bool(int(os.environ.get("KERNEL_TRACE", "0")))
    res = run_bass_kernel_spmd(nc, in_maps[:ncores_run],
                               core_ids=list(range(ncores_run)), trace=trace)
    _LAST_RESULTS = {"exec_time_ns": res.exec_time_ns,
                     "profile_json": res.profile_json,
                     "res": res}

    out = np.empty((B, T, D), np.float32)
    for c in range(NCORES):
        b, qh = c // 2, c % 2
        out[b, qh * TQ:(qh + 1) * TQ, :] = res.results[c % ncores_run]["out"]
    return out
